# revision 21
# baseline (speedup 1.0000x reference)
"""KAN layer (uniform cubic B-spline, grid=8, k=3) Trainium2 kernel, v4.

Math (unchanged from v3)
------------------------
Per batch row n and output o:
    out[n,o] = sum_i w_silu[i,o]*silu(x[n,i]) + sum_i w_sp[i,o] * sum_b B_b(x[n,i]) * C[b,i,o]

With the uniform knot grid t_j = -1.75 + 0.25*j, put s = 4x+7 in [3,11). The
spline space restricted to [-1,1] is spanned by 11 truncated-power functions
{1, x, x^2, x^3} u {(s-k)_+^3/6 : k=7..10} u {(k-s)_+^3/6 : k=4..6} (the
two-sided split keeps every tile O(1)-bounded so reduced-precision matmul
stays well-conditioned). silu(x) is least-squares fitted in the SAME basis
(max abs fit err 2.9e-6) so w_silu folds into the 11 weight groups. The
device computes 11 activation tiles and contracts them against (n_in, n_out)
weight groups on the PE, fp32 PSUM. Precision mix (host-sim rel err 1.31e-2
vs 2e-2 budget): P0/P1 bf16, P2/P3/R7 f32r, L6/R8/L5/R9 bf16, L4+R10 packed
in ONE fp8e4m3 DoubleRow group (tiles 2^10, weights 2^12, second PSUM bank
set, merged as out = psum_main + 2^-22 * psum_fp8).

v4 schedule changes (v3 measured: ACT 33.4us busy vs PE-stream 33us; PE idle
0..7.9us; 1KB-line output DMAs issued late through a backed-up Sync queue)
---------------------------------------------------------------------------
- Each cube tile is now Square(ACT, scale/bias does the knot shift; the *4
  of u=s-k folded into the scale) -> stt(DVE, (x+(7-k)/4)*t2) -> relu as a
  DVE tensor_scalar max(q,0) [R side] / min(q,0) [L side, weights negated on
  host] - the 7 ACT Relu passes (14us) move to DVE at 0.6us each, and the
  f32 U=4x+7 helper tile disappears (stt reads xt directly).
- p2 / p3 are single 2048-wide passes instead of 4x512.
- PE warm-up: the HAM clock gate needs ~3.4us of continuous PE work before
  it unthrottles 1.2->2.4 GHz; scratch matmuls + the 4 rank-1 psum seeds
  fill the DMA lead-in exactly, so real groups run at full clock.
- Group order P1,P2,P3,R7,L6,R8,L5,DR,R9: the fp8 group closes early so its
  ACT scale-copies overlap R9; R9 closes each psum bank last and its per-m
  completion chases a full-width DVE merge + one 2KB-line output DMA.
- One dma_start per weight plane, separate SBUF tiles per plane (exact
  dependency granularity), issued in first-use order.

Sharding: data-parallel over batch N across 8 cores (512 rows each);
weights replicated. No collectives.
"""

import numpy as np
import ml_dtypes

N, N_IN, N_OUT = 4096, 512, 512
NB = 11
NCORES = 8
ROWS = N // NCORES          # batch rows per core
G = N_IN // 128             # 4 partition groups over n_in
M = ROWS // 128             # 4 PSUM row-chunks
W5 = (1.0, -4.0, 6.0, -4.0, 1.0)

# silu(x) ~ lstsq fit in the truncated-power basis
# [1, x, x^2, x^3, L4, L5, L6, R7, R8, R9, R10]  (max abs err 2.9e-6)
SILU_COEF = np.array([
    -2.85017504e-06, 5.00000000e-01, 2.51316134e-01, 1.04215478e-02,
    -6.77741053e-04, -1.30248882e-03, -1.77424080e-03, -1.95404022e-03,
    -1.77424080e-03, -1.30248882e-03, -6.77741053e-04,
])

# full-group cubes in matmul-group order: (kind, knot, chain dtype)
CUBES_R = [("R", 7, "f32"), ("L", 6, "bf"), ("R", 8, "bf"), ("L", 5, "bf"), ("R", 9, "bf")]
CUBES_8 = [("L", 4), ("R", 10)]   # fp8 DoubleRow pair
A_SH = 10   # fp8 tile scale 2^A_SH
B_SH = 12   # fp8 weight scale 2^B_SH
N_SCRATCH = 50   # scratch warm-up matmuls before the 4 psum seeds

_CACHE = {}


def _fp32r(a):
    """Round float32 array to fp32r (11-bit mantissa, RNE)."""
    a = np.ascontiguousarray(a, dtype=np.float32)
    bits = a.view(np.uint32)
    rnd = ((bits >> np.uint32(12)) & np.uint32(1)) + np.uint32(0x7FF)
    return ((bits + rnd) & np.uint32(0xFFFFF000)).view(np.float32)


def _poly_alpha():
    """alpha[j, t]: coefficient of x^t in the polynomial part of B_j."""
    alpha = np.zeros((NB, 4), dtype=np.float64)
    for j in range(NB):
        for p in range(5):
            k = j + p
            if k <= 6:  # (s-k)^3/6 with s-k = 4x + (7-k)
                a = 7.0 - k
                alpha[j, 3] += W5[p] * 64.0 / 6.0
                alpha[j, 2] += W5[p] * 48.0 * a / 6.0
                alpha[j, 1] += W5[p] * 12.0 * a * a / 6.0
                alpha[j, 0] += W5[p] * a * a * a / 6.0
    return alpha


def _prep_weights(C, w_silu, w_sp):
    """Fold C*w_sp and the silu fit into the 11 weight groups.
    L-side cube tiles are produced as min(q,0) = -(k-s)_+^3/6 on device,
    so L weights are negated here."""
    Ceff = C.astype(np.float64) * w_sp.astype(np.float64)[None]
    ws = w_silu.astype(np.float64)
    alpha = _poly_alpha()
    beta = np.einsum("jt,jio->tio", alpha, Ceff)  # (4, n_in, n_out)
    Wp = [beta[t] + SILU_COEF[t] * ws for t in range(4)]

    cube_order = [("L", 4), ("L", 5), ("L", 6), ("R", 7), ("R", 8), ("R", 9), ("R", 10)]
    Wc = {}
    for gi, (kind, k) in enumerate(cube_order):
        wk = np.zeros((N_IN, N_OUT), dtype=np.float64)
        for p in range(5):
            j = k - p
            if 0 <= j < NB:
                wk += W5[p] * Ceff[j]
        wk = wk + SILU_COEF[4 + gi] * ws
        if kind == "L":
            wk = -wk          # device L tiles are min(q,0) <= 0
        Wc[(kind, k)] = wk

    bf = ml_dtypes.bfloat16
    BROW = Wp[0].sum(axis=0, keepdims=True).astype(np.float32).astype(bf)
    WP1 = Wp[1].astype(np.float32).astype(bf)
    WF = _fp32r(np.stack([Wp[2], Wp[3], Wc[("R", 7)]]).astype(np.float32))
    WB = np.stack([Wc[("L", 6)], Wc[("R", 8)], Wc[("L", 5)], Wc[("R", 9)]])
    WB = WB.astype(np.float32).astype(bf)
    W8 = np.empty((N_IN, 2, N_OUT), dtype=np.float32)
    W8[:, 0] = Wc[CUBES_8[0]] * 2.0 ** B_SH
    W8[:, 1] = Wc[CUBES_8[1]] * 2.0 ** B_SH
    W8 = W8.astype(ml_dtypes.float8_e4m3)
    return BROW, WP1, WF, WB, W8


def _build():
    import concourse.bacc as bacc
    import concourse.mybir as mybir
    from concourse import tile

    f32 = mybir.dt.float32
    f32r = mybir.dt.float32r
    bf16 = mybir.dt.bfloat16
    fp8 = mybir.dt.float8e4
    AF = mybir.ActivationFunctionType
    ALU = mybir.AluOpType
    DR = mybir.MatmulPerfMode.DoubleRow

    c2 = 6.0 ** (-0.5)          # sqrt scaling: t2 = (2*c2*(s-k))^2 = (2/3)u^2
    sA = 2.0 ** (A_SH / 2)      # extra sqrt scale for the fp8 pair

    nc = bacc.Bacc("TRN2", target_bir_lowering=False, debug=False)
    TXT = nc.dram_tensor("xT", [N_IN, ROWS], bf16, kind="ExternalInput").ap()
    TP1 = nc.dram_tensor("Wp1", [N_IN, N_OUT], bf16, kind="ExternalInput").ap()
    TB = nc.dram_tensor("Brow", [128, N_OUT], bf16, kind="ExternalInput").ap()
    NBIAS = len(CUBES_R) + len(CUBES_8)
    TBIAS = nc.dram_tensor("Bias", [128, NBIAS], f32, kind="ExternalInput").ap()
    TWF = [nc.dram_tensor(f"Wf{i}", [N_IN, N_OUT], f32r,
                          kind="ExternalInput").ap() for i in range(3)]
    TWB01 = nc.dram_tensor("Wb01", [2, N_IN, N_OUT], bf16, kind="ExternalInput").ap()
    TWB23 = nc.dram_tensor("Wb23", [2, N_IN, N_OUT], bf16, kind="ExternalInput").ap()
    TW8 = nc.dram_tensor("W8", [N_IN, 2, N_OUT], fp8, kind="ExternalInput").ap()
    OUT = nc.dram_tensor("out", [ROWS, N_OUT], f32, kind="ExternalOutput").ap()

    with tile.TileContext(nc) as tc:
        with (
            tc.tile_pool(name="const", bufs=1) as constp,
            tc.tile_pool(name="t2f", bufs=1) as t2fp,
            tc.tile_pool(name="t2b", bufs=3) as t2bp,
            tc.tile_pool(name="qf", bufs=1) as qfp,
            tc.tile_pool(name="qb", bufs=3) as qbp,
            tc.tile_pool(name="outp", bufs=6) as outp,
            tc.tile_pool(name="psp", bufs=1, space="PSUM") as psp,
        ):
            # ---- persistent tiles ----
            xt = constp.tile([128, G, ROWS], bf16)
            dqP1 = constp.tile([128, G, N_OUT], bf16)
            wrm = constp.tile([128, 128], mybir.dt.uint16)
            brow2 = constp.tile([128, N_OUT], bf16)   # brow/128 replicated
            bias_t2 = constp.tile([128, NBIAS], f32)
            dqF = [constp.tile([128, G, N_OUT], f32r, name=f"dqF{i}",
                                tag=f"dqF{i}") for i in range(3)]
            dqB01 = constp.tile([128, 2, G, N_OUT], bf16)
            dqB23 = constp.tile([128, 2, G, N_OUT], bf16)
            dq8 = constp.tile([128, G, 2, N_OUT], fp8)
            p2 = constp.tile([128, G, ROWS], f32r)
            p3 = constp.tile([128, G, ROWS], f32r)
            cube_tiles = []
            for ci, (kind, k, dt) in enumerate(CUBES_R):
                cube_tiles.append(constp.tile(
                    [128, G, ROWS], f32r if dt == "f32" else bf16,
                    name=f"cube{k}", tag=f"cube{k}"))
            cube8 = constp.tile([128, 2, G, ROWS], fp8)

            # ---- all DMAs up-front, bundled, first-use order.  brow is
            # FIRST: transfers drain FIFO, and the psum seed matmuls (which
            # precede P1 on the PE queue) block on it ----
            # bf16 memset fails the walrus ISA check; memset the bf16 1.0
            # bit-pattern into a uint16 tile and bitcast for the warm-ups
            nc.gpsimd.memset(wrm[:], 0x3F80)
            nc.sync.dma_start(brow2[:], TB[:])
            nc.sync.dma_start(xt[:], TXT.rearrange("(g p) n -> p g n", p=128))
            nc.sync.dma_start(dqP1[:], TP1.rearrange("(g p) o -> p g o", p=128))
            nc.sync.dma_start(bias_t2[:], TBIAS[:])
            for i in range(3):
                nc.sync.dma_start(dqF[i][:], TWF[i].rearrange("(g p) o -> p g o", p=128))
            nc.sync.dma_start(dqB01[:], TWB01.rearrange("t (g p) o -> p t g o", p=128))
            nc.sync.dma_start(dqB23[:], TWB23.rearrange("t (g p) o -> p t g o", p=128))
            nc.sync.dma_start(dq8[:], TW8.rearrange("(g p) two o -> p g two o", p=128))

            ot_all = constp.tile([128, M, N_OUT], f32)
            psm = [psp.tile([128, N_OUT], f32, name=f"ps{m}", tag=f"ps{m}") for m in range(M)]
            ps8 = [psp.tile([128, N_OUT], f32, name=f"q{m}", tag=f"q{m}") for m in range(M)]

            # ---- PE warm-up: fill the HAM window until Wp1's DMA lands.
            # Full 128-K matmuls: HAM's activity monitor ignores rank-1 work.
            # N=128 -> ~107ns each cold; the tiny wrm block lands first ----
            for _ in range(N_SCRATCH):
                nc.tensor.matmul(ps8[M - 1][:, 0:128], wrm[:].bitcast(bf16),
                                 wrm[:].bitcast(bf16), start=True, stop=True)
            # constant group seeds each psum bank: full-K ones x brow/128
            # (K=128 so the HAM activity monitor credits these too)
            for m in range(M):
                nc.tensor.matmul(psm[m][:], wrm[:].bitcast(bf16), brow2[:],
                                 start=True, stop=False)

            # ---- ACT queue: p2 then one Square per cube, in group order ----
            nc.scalar.activation(p2[:], xt[:], AF.Square)
            sq = {}
            for ci, (kind, k, dt) in enumerate(CUBES_R):
                pool = t2fp if dt == "f32" else t2bp
                t2 = pool.tile([128, G, ROWS], f32 if dt == "f32" else bf16,
                               name="t2", tag=f"t2{dt}")
                nc.scalar.activation(t2[:], xt[:], AF.Square,
                                     bias=bias_t2[:, ci:ci + 1], scale=8.0 * c2)
                sq[ci] = t2
            for idx, (kind, k) in enumerate(CUBES_8):
                t2 = t2bp.tile([128, G, ROWS], bf16, name="t2", tag="t2bf")
                nc.scalar.activation(t2[:], xt[:], AF.Square,
                                     bias=bias_t2[:, 5 + idx:5 + idx + 1],
                                     scale=8.0 * c2 * sA)
                sq[5 + idx] = t2

            # ---- DVE queue: p3, then stt+relu per cube, in group order ----
            nc.vector.tensor_tensor(p3[:], p2[:], xt[:], op=ALU.mult)

            def emit_cube(kind, k, t2, dst, dt):
                pool = qfp if dt == "f32" else qbp
                q = pool.tile([128, G, ROWS], f32 if dt == "f32" else bf16,
                              name="q", tag=f"q{dt}")
                # q = (x + (7-k)/4) * t2 = u^3/6 (t2 carries the 4x scale)
                nc.vector.scalar_tensor_tensor(q[:], xt[:], (7.0 - k) / 4.0, t2[:],
                                               op0=ALU.add, op1=ALU.mult)
                # R: relu(q) = max(q,0); L: min(q,0) = -(k-s)_+^3/6 (W negated)
                nc.vector.tensor_scalar(dst, q[:], 0.0, None,
                                        op0=ALU.max if kind == "R" else ALU.min)

            for ci, (kind, k, dt) in enumerate(CUBES_R):
                emit_cube(kind, k, sq[ci], cube_tiles[ci][:], dt)
            for idx, (kind, k) in enumerate(CUBES_8):
                emit_cube(kind, k, sq[5 + idx], cube8[:, idx], "bf")

            # ---- matmul groups ----
            def emit_mm(lhs_of, dq_of, last=False):
                for m in range(M):
                    for g in range(G):
                        nc.tensor.matmul(
                            psm[m][:], lhs_of(m, g), dq_of(g),
                            start=False, stop=(last and g == G - 1),
                        )

            emit_mm(lambda m, g: xt[:, g, m * 128:(m + 1) * 128],
                    lambda g: dqP1[:, g, :])
            emit_mm(lambda m, g: p2[:, g, m * 128:(m + 1) * 128],
                    lambda g: dqF[0][:, g, :])
            emit_mm(lambda m, g: p3[:, g, m * 128:(m + 1) * 128],
                    lambda g: dqF[1][:, g, :])
            dq_of = [lambda g: dqF[2][:, g, :], lambda g: dqB01[:, 0, g, :],
                     lambda g: dqB01[:, 1, g, :], lambda g: dqB23[:, 0, g, :],
                     lambda g: dqB23[:, 1, g, :]]
            for ci in range(len(CUBES_R) - 1):       # R7, L6, R8, L5
                cube = cube_tiles[ci]
                emit_mm(lambda m, g, cube=cube: cube[:, g, m * 128:(m + 1) * 128],
                        dq_of[ci])

            # fp8 DoubleRow pair into the second PSUM bank set; scale-copies
            # to SBUF chase each m so the R9 merges only touch one PSUM tile
            tmp8s = []
            for m in range(M):
                for g in range(G):
                    nc.tensor.matmul(
                        ps8[m][:],
                        cube8[:, :, g, m * 128:(m + 1) * 128],
                        dq8[:, g, :, :],
                        start=(g == 0), stop=(g == G - 1),
                        perf_mode=DR,
                    )
                tmp8 = outp.tile([128, N_OUT], f32, name="tmp8", tag="tmp8")
                nc.scalar.activation(tmp8[:], ps8[m][:], AF.Copy,
                                     scale=2.0 ** -(A_SH + B_SH))
                tmp8s.append(tmp8)

            # last bf16 group (R9) closes the main accumulation; each m's
            # merge + full-width store chases its final matmul
            cube = cube_tiles[len(CUBES_R) - 1]
            for m in range(M):
                for g in range(G):
                    nc.tensor.matmul(
                        psm[m][:], cube[:, g, m * 128:(m + 1) * 128],
                        dq_of[len(CUBES_R) - 1](g),
                        start=False, stop=(g == G - 1),
                    )
                nc.vector.scalar_tensor_tensor(
                    ot_all[:, m, :], psm[m][:], 1.0, tmp8s[m][:],
                    op0=ALU.mult, op1=ALU.add,
                )
                nc.sync.dma_start(OUT[m * 128:(m + 1) * 128, :], ot_all[:, m, :])

    nc.compile()
    return nc


# test-harness knobs (the grader just calls kernel())
TRACE = False
LAST_RESULTS = None


def kernel(x, grid, C, w_silu, w_sp):
    from concourse import bass_utils

    if "nc" not in _CACHE:
        _CACHE["nc"] = _build()
    nc = _CACHE["nc"]

    x = np.ascontiguousarray(np.asarray(x, dtype=np.float32))
    BROW, WP1, WF, WB, W8 = _prep_weights(np.asarray(C), np.asarray(w_silu),
                                          np.asarray(w_sp))

    WP1bf = WP1.astype(ml_dtypes.bfloat16)
    bf = ml_dtypes.bfloat16
    BROW2 = np.tile((BROW.astype(np.float32) / 128.0).astype(bf), (128, 1))
    c2 = 6.0 ** -0.5
    sA = 2.0 ** (A_SH / 2)
    bias_vals = [2.0 * (7.0 - k) * c2 for _, k, _d in CUBES_R] + \
                [2.0 * (7.0 - k) * c2 * sA for _, k in CUBES_8]
    BIAS = np.tile(np.array(bias_vals, dtype=np.float32)[None, :], (128, 1))
    in_maps = []
    for c in range(NCORES):
        xT = np.ascontiguousarray(x[c * ROWS:(c + 1) * ROWS].T).astype(ml_dtypes.bfloat16)
        in_maps.append({"xT": xT, "Wp1": WP1bf, "Brow": BROW2,
                        "Bias": BIAS, "Wf0": WF[0], "Wf1": WF[1], "Wf2": WF[2],
                        "Wb01": WB[0:2], "Wb23": WB[2:4], "W8": W8})

    res = bass_utils.run_bass_kernel_spmd(
        nc, in_maps, core_ids=list(range(NCORES)), trace=TRACE
    )
    global LAST_RESULTS
    LAST_RESULTS = res
    return np.concatenate([res.results[c]["out"] for c in range(NCORES)], axis=0)


# revision 22
# speedup vs baseline: 1.0014x; 1.0014x over previous
"""KAN layer (uniform cubic B-spline, grid=8, k=3) Trainium2 kernel, v4.

Math (unchanged from v3)
------------------------
Per batch row n and output o:
    out[n,o] = sum_i w_silu[i,o]*silu(x[n,i]) + sum_i w_sp[i,o] * sum_b B_b(x[n,i]) * C[b,i,o]

With the uniform knot grid t_j = -1.75 + 0.25*j, put s = 4x+7 in [3,11). The
spline space restricted to [-1,1] is spanned by 11 truncated-power functions
{1, x, x^2, x^3} u {(s-k)_+^3/6 : k=7..10} u {(k-s)_+^3/6 : k=4..6} (the
two-sided split keeps every tile O(1)-bounded so reduced-precision matmul
stays well-conditioned). silu(x) is least-squares fitted in the SAME basis
(max abs fit err 2.9e-6) so w_silu folds into the 11 weight groups. The
device computes 11 activation tiles and contracts them against (n_in, n_out)
weight groups on the PE, fp32 PSUM. Precision mix (host-sim rel err 1.31e-2
vs 2e-2 budget): P0/P1 bf16, P2/P3/R7 f32r, L6/R8/L5/R9 bf16, L4+R10 packed
in ONE fp8e4m3 DoubleRow group (tiles 2^10, weights 2^12, second PSUM bank
set, merged as out = psum_main + 2^-22 * psum_fp8).

v4 schedule changes (v3 measured: ACT 33.4us busy vs PE-stream 33us; PE idle
0..7.9us; 1KB-line output DMAs issued late through a backed-up Sync queue)
---------------------------------------------------------------------------
- Each cube tile is now Square(ACT, scale/bias does the knot shift; the *4
  of u=s-k folded into the scale) -> stt(DVE, (x+(7-k)/4)*t2) -> relu as a
  DVE tensor_scalar max(q,0) [R side] / min(q,0) [L side, weights negated on
  host] - the 7 ACT Relu passes (14us) move to DVE at 0.6us each, and the
  f32 U=4x+7 helper tile disappears (stt reads xt directly).
- p2 / p3 are single 2048-wide passes instead of 4x512.
- PE warm-up: the HAM clock gate needs ~3.4us of continuous PE work before
  it unthrottles 1.2->2.4 GHz; scratch matmuls + the 4 rank-1 psum seeds
  fill the DMA lead-in exactly, so real groups run at full clock.
- Group order P1,P2,P3,R7,L6,R8,L5,DR,R9: the fp8 group closes early so its
  ACT scale-copies overlap R9; R9 closes each psum bank last and its per-m
  completion chases a full-width DVE merge + one 2KB-line output DMA.
- One dma_start per weight plane, separate SBUF tiles per plane (exact
  dependency granularity), issued in first-use order.

Sharding: data-parallel over batch N across 8 cores (512 rows each);
weights replicated. No collectives.
"""

import numpy as np
import ml_dtypes

N, N_IN, N_OUT = 4096, 512, 512
NB = 11
NCORES = 8
ROWS = N // NCORES          # batch rows per core
G = N_IN // 128             # 4 partition groups over n_in
M = ROWS // 128             # 4 PSUM row-chunks
W5 = (1.0, -4.0, 6.0, -4.0, 1.0)

# silu(x) ~ lstsq fit in the truncated-power basis
# [1, x, x^2, x^3, L4, L5, L6, R7, R8, R9, R10]  (max abs err 2.9e-6)
SILU_COEF = np.array([
    -2.85017504e-06, 5.00000000e-01, 2.51316134e-01, 1.04215478e-02,
    -6.77741053e-04, -1.30248882e-03, -1.77424080e-03, -1.95404022e-03,
    -1.77424080e-03, -1.30248882e-03, -6.77741053e-04,
])

# full-group cubes in matmul-group order: (kind, knot, chain dtype)
CUBES_R = [("R", 7, "f32"), ("L", 6, "bf"), ("R", 8, "bf"), ("L", 5, "bf"), ("R", 9, "bf")]
CUBES_8 = [("L", 4), ("R", 10)]   # fp8 DoubleRow pair
A_SH = 10   # fp8 tile scale 2^A_SH
B_SH = 12   # fp8 weight scale 2^B_SH
N_SCRATCH = 50   # scratch warm-up matmuls before the 4 psum seeds

_CACHE = {}


def _fp32r(a):
    """Round float32 array to fp32r (11-bit mantissa, RNE)."""
    a = np.ascontiguousarray(a, dtype=np.float32)
    bits = a.view(np.uint32)
    rnd = ((bits >> np.uint32(12)) & np.uint32(1)) + np.uint32(0x7FF)
    return ((bits + rnd) & np.uint32(0xFFFFF000)).view(np.float32)


def _poly_alpha():
    """alpha[j, t]: coefficient of x^t in the polynomial part of B_j."""
    alpha = np.zeros((NB, 4), dtype=np.float64)
    for j in range(NB):
        for p in range(5):
            k = j + p
            if k <= 6:  # (s-k)^3/6 with s-k = 4x + (7-k)
                a = 7.0 - k
                alpha[j, 3] += W5[p] * 64.0 / 6.0
                alpha[j, 2] += W5[p] * 48.0 * a / 6.0
                alpha[j, 1] += W5[p] * 12.0 * a * a / 6.0
                alpha[j, 0] += W5[p] * a * a * a / 6.0
    return alpha


def _prep_weights(C, w_silu, w_sp):
    """Fold C*w_sp and the silu fit into the 11 weight groups.
    L-side cube tiles are produced as min(q,0) = -(k-s)_+^3/6 on device,
    so L weights are negated here."""
    Ceff = C.astype(np.float64) * w_sp.astype(np.float64)[None]
    ws = w_silu.astype(np.float64)
    alpha = _poly_alpha()
    beta = np.einsum("jt,jio->tio", alpha, Ceff)  # (4, n_in, n_out)
    Wp = [beta[t] + SILU_COEF[t] * ws for t in range(4)]

    cube_order = [("L", 4), ("L", 5), ("L", 6), ("R", 7), ("R", 8), ("R", 9), ("R", 10)]
    Wc = {}
    for gi, (kind, k) in enumerate(cube_order):
        wk = np.zeros((N_IN, N_OUT), dtype=np.float64)
        for p in range(5):
            j = k - p
            if 0 <= j < NB:
                wk += W5[p] * Ceff[j]
        wk = wk + SILU_COEF[4 + gi] * ws
        if kind == "L":
            wk = -wk          # device L tiles are min(q,0) <= 0
        Wc[(kind, k)] = wk

    bf = ml_dtypes.bfloat16
    BROW = Wp[0].sum(axis=0, keepdims=True).astype(np.float32).astype(bf)
    WP1 = Wp[1].astype(np.float32).astype(bf)
    WF = _fp32r(np.stack([Wp[2], Wp[3], Wc[("R", 7)]]).astype(np.float32))
    WB = np.stack([Wc[("L", 6)], Wc[("R", 8)], Wc[("L", 5)], Wc[("R", 9)]])
    WB = WB.astype(np.float32).astype(bf)
    W8 = np.empty((N_IN, 2, N_OUT), dtype=np.float32)
    W8[:, 0] = Wc[CUBES_8[0]] * 2.0 ** B_SH
    W8[:, 1] = Wc[CUBES_8[1]] * 2.0 ** B_SH
    W8 = W8.astype(ml_dtypes.float8_e4m3)
    return BROW, WP1, WF, WB, W8


def _build():
    import concourse.bacc as bacc
    import concourse.mybir as mybir
    from concourse import tile

    f32 = mybir.dt.float32
    f32r = mybir.dt.float32r
    bf16 = mybir.dt.bfloat16
    fp8 = mybir.dt.float8e4
    AF = mybir.ActivationFunctionType
    ALU = mybir.AluOpType
    DR = mybir.MatmulPerfMode.DoubleRow

    c2 = 6.0 ** (-0.5)          # sqrt scaling: t2 = (2*c2*(s-k))^2 = (2/3)u^2
    sA = 2.0 ** (A_SH / 2)      # extra sqrt scale for the fp8 pair

    nc = bacc.Bacc("TRN2", target_bir_lowering=False, debug=False)
    TXT = nc.dram_tensor("xT", [N_IN, ROWS], bf16, kind="ExternalInput").ap()
    TP1 = nc.dram_tensor("Wp1", [N_IN, N_OUT], bf16, kind="ExternalInput").ap()
    TB = nc.dram_tensor("Brow", [128, N_OUT], bf16, kind="ExternalInput").ap()
    NBIAS = len(CUBES_R) + len(CUBES_8)
    TBIAS = nc.dram_tensor("Bias", [128, NBIAS], f32, kind="ExternalInput").ap()
    TWF = [nc.dram_tensor(f"Wf{i}", [N_IN, N_OUT], f32r,
                          kind="ExternalInput").ap() for i in range(3)]
    TWB01 = nc.dram_tensor("Wb01", [2, N_IN, N_OUT], bf16, kind="ExternalInput").ap()
    TWB23 = nc.dram_tensor("Wb23", [2, N_IN, N_OUT], bf16, kind="ExternalInput").ap()
    TW8 = nc.dram_tensor("W8", [N_IN, 2, N_OUT], fp8, kind="ExternalInput").ap()
    OUT = nc.dram_tensor("out", [ROWS, N_OUT], f32, kind="ExternalOutput").ap()

    with tile.TileContext(nc) as tc:
        with (
            tc.tile_pool(name="const", bufs=1) as constp,
            tc.tile_pool(name="t2f", bufs=1) as t2fp,
            tc.tile_pool(name="t2b", bufs=3) as t2bp,
            tc.tile_pool(name="qf", bufs=1) as qfp,
            tc.tile_pool(name="qb", bufs=3) as qbp,
            tc.tile_pool(name="outp", bufs=6) as outp,
            tc.tile_pool(name="psp", bufs=1, space="PSUM") as psp,
        ):
            # ---- persistent tiles ----
            xt = constp.tile([128, G, ROWS], bf16)
            dqP1 = constp.tile([128, G, N_OUT], bf16)
            wrm = constp.tile([128, 128], mybir.dt.uint16)
            brow2 = constp.tile([128, N_OUT], bf16)   # brow/128 replicated
            bias_t2 = constp.tile([128, NBIAS], f32)
            dqF = [constp.tile([128, G, N_OUT], f32r, name=f"dqF{i}",
                                tag=f"dqF{i}") for i in range(3)]
            dqB01 = constp.tile([128, 2, G, N_OUT], bf16)
            dqB23 = constp.tile([128, 2, G, N_OUT], bf16)
            dq8 = constp.tile([128, G, 2, N_OUT], fp8)
            p2 = constp.tile([128, G, ROWS], f32r)
            p3 = constp.tile([128, G, ROWS], f32r)
            cube_tiles = []
            for ci, (kind, k, dt) in enumerate(CUBES_R):
                cube_tiles.append(constp.tile(
                    [128, G, ROWS], f32r if dt == "f32" else bf16,
                    name=f"cube{k}", tag=f"cube{k}"))
            cube8 = constp.tile([128, 2, G, ROWS], fp8)

            # ---- all DMAs up-front, bundled, first-use order.  brow is
            # FIRST: transfers drain FIFO, and the psum seed matmuls (which
            # precede P1 on the PE queue) block on it ----
            # bf16 memset fails the walrus ISA check; memset the bf16 1.0
            # bit-pattern into a uint16 tile and bitcast for the warm-ups
            nc.gpsimd.memset(wrm[:], 0x3F80)
            nc.sync.dma_start(brow2[:], TB[:])
            nc.sync.dma_start(xt[:], TXT.rearrange("(g p) n -> p g n", p=128))
            nc.sync.dma_start(dqP1[:], TP1.rearrange("(g p) o -> p g o", p=128))
            nc.sync.dma_start(bias_t2[:], TBIAS[:])
            for i in range(3):
                nc.sync.dma_start(dqF[i][:], TWF[i].rearrange("(g p) o -> p g o", p=128))
            nc.sync.dma_start(dq8[:], TW8.rearrange("(g p) two o -> p g two o", p=128))
            nc.sync.dma_start(dqB01[:], TWB01.rearrange("t (g p) o -> p t g o", p=128))
            nc.sync.dma_start(dqB23[:], TWB23.rearrange("t (g p) o -> p t g o", p=128))

            ot_all = constp.tile([128, M, N_OUT], f32)
            psm = [psp.tile([128, N_OUT], f32, name=f"ps{m}", tag=f"ps{m}") for m in range(M)]
            ps8 = [psp.tile([128, N_OUT], f32, name=f"q{m}", tag=f"q{m}") for m in range(M)]

            # ---- PE warm-up: fill the HAM window until Wp1's DMA lands.
            # Full 128-K matmuls: HAM's activity monitor ignores rank-1 work.
            # N=128 -> ~107ns each cold; the tiny wrm block lands first ----
            for _ in range(N_SCRATCH):
                nc.tensor.matmul(ps8[M - 1][:, 0:128], wrm[:].bitcast(bf16),
                                 wrm[:].bitcast(bf16), start=True, stop=True)
            # constant group seeds each psum bank: full-K ones x brow/128
            # (K=128 so the HAM activity monitor credits these too)
            for m in range(M):
                nc.tensor.matmul(psm[m][:], wrm[:].bitcast(bf16), brow2[:],
                                 start=True, stop=False)

            # ---- ACT queue: p2 then one Square per cube, in group order ----
            nc.scalar.activation(p2[:], xt[:], AF.Square)
            sq = {}

            def emit_sq(ci):
                kind, k, dt = CUBES_R[ci]
                pool = t2fp if dt == "f32" else t2bp
                t2 = pool.tile([128, G, ROWS], f32 if dt == "f32" else bf16,
                               name="t2", tag=f"t2{dt}")
                nc.scalar.activation(t2[:], xt[:], AF.Square,
                                     bias=bias_t2[:, ci:ci + 1], scale=8.0 * c2)
                sq[ci] = t2

            def emit_sq8(idx):
                t2 = t2bp.tile([128, G, ROWS], bf16, name="t2", tag="t2bf")
                nc.scalar.activation(t2[:], xt[:], AF.Square,
                                     bias=bias_t2[:, 5 + idx:5 + idx + 1],
                                     scale=8.0 * c2 * sA)
                sq[5 + idx] = t2

            emit_sq(0)            # R7 chain leads: its group follows P3
            emit_sq8(0)           # then the fp8 pair (DR group runs 5th)
            emit_sq8(1)
            for ci in (1, 2, 3, 4):
                emit_sq(ci)

            # ---- DVE queue: p3, then stt+relu per cube, in group order ----
            nc.vector.tensor_tensor(p3[:], p2[:], xt[:], op=ALU.mult)

            def emit_cube(kind, k, t2, dst, dt):
                pool = qfp if dt == "f32" else qbp
                q = pool.tile([128, G, ROWS], f32 if dt == "f32" else bf16,
                              name="q", tag=f"q{dt}")
                # q = (x + (7-k)/4) * t2 = u^3/6 (t2 carries the 4x scale)
                nc.vector.scalar_tensor_tensor(q[:], xt[:], (7.0 - k) / 4.0, t2[:],
                                               op0=ALU.add, op1=ALU.mult)
                # R: relu(q) = max(q,0); L: min(q,0) = -(k-s)_+^3/6 (W negated)
                nc.vector.tensor_scalar(dst, q[:], 0.0, None,
                                        op0=ALU.max if kind == "R" else ALU.min)

            emit_cube(*CUBES_R[0][:2], sq[0], cube_tiles[0][:], CUBES_R[0][2])
            for idx, (kind, k) in enumerate(CUBES_8):
                emit_cube(kind, k, sq[5 + idx], cube8[:, idx], "bf")
            for ci in (1, 2, 3, 4):
                kind, k, dt = CUBES_R[ci]
                emit_cube(kind, k, sq[ci], cube_tiles[ci][:], dt)

            # ---- matmul groups ----
            def emit_mm(lhs_of, dq_of, last=False):
                for m in range(M):
                    for g in range(G):
                        nc.tensor.matmul(
                            psm[m][:], lhs_of(m, g), dq_of(g),
                            start=False, stop=(last and g == G - 1),
                        )

            emit_mm(lambda m, g: xt[:, g, m * 128:(m + 1) * 128],
                    lambda g: dqP1[:, g, :])
            emit_mm(lambda m, g: p2[:, g, m * 128:(m + 1) * 128],
                    lambda g: dqF[0][:, g, :])
            emit_mm(lambda m, g: p3[:, g, m * 128:(m + 1) * 128],
                    lambda g: dqF[1][:, g, :])
            dq_of = [lambda g: dqF[2][:, g, :], lambda g: dqB01[:, 0, g, :],
                     lambda g: dqB01[:, 1, g, :], lambda g: dqB23[:, 0, g, :],
                     lambda g: dqB23[:, 1, g, :]]
            emit_mm(lambda m, g: cube_tiles[0][:, g, m * 128:(m + 1) * 128],
                    dq_of[0])                        # R7

            # fp8 DoubleRow pair into the second PSUM bank set; scale-copies
            # to SBUF chase each m so the R9 merges only touch one PSUM tile
            tmp8s = []
            for m in range(M):
                for g in range(G):
                    nc.tensor.matmul(
                        ps8[m][:],
                        cube8[:, :, g, m * 128:(m + 1) * 128],
                        dq8[:, g, :, :],
                        start=(g == 0), stop=(g == G - 1),
                        perf_mode=DR,
                    )
                tmp8 = outp.tile([128, N_OUT], f32, name="tmp8", tag="tmp8")
                nc.scalar.activation(tmp8[:], ps8[m][:], AF.Copy,
                                     scale=2.0 ** -(A_SH + B_SH))
                tmp8s.append(tmp8)

            for ci in (1, 2, 3):                     # L6, R8, L5
                cube = cube_tiles[ci]
                emit_mm(lambda m, g, cube=cube: cube[:, g, m * 128:(m + 1) * 128],
                        dq_of[ci])

            # last bf16 group (R9) closes the main accumulation; each m's
            # merge + full-width store chases its final matmul
            cube = cube_tiles[len(CUBES_R) - 1]
            for m in range(M):
                for g in range(G):
                    nc.tensor.matmul(
                        psm[m][:], cube[:, g, m * 128:(m + 1) * 128],
                        dq_of[len(CUBES_R) - 1](g),
                        start=False, stop=(g == G - 1),
                    )
                nc.vector.scalar_tensor_tensor(
                    ot_all[:, m, :], psm[m][:], 1.0, tmp8s[m][:],
                    op0=ALU.mult, op1=ALU.add,
                )
                nc.sync.dma_start(OUT[m * 128:(m + 1) * 128, :], ot_all[:, m, :])

    nc.compile()
    return nc


# test-harness knobs (the grader just calls kernel())
TRACE = False
LAST_RESULTS = None


def kernel(x, grid, C, w_silu, w_sp):
    from concourse import bass_utils

    if "nc" not in _CACHE:
        _CACHE["nc"] = _build()
    nc = _CACHE["nc"]

    x = np.ascontiguousarray(np.asarray(x, dtype=np.float32))
    BROW, WP1, WF, WB, W8 = _prep_weights(np.asarray(C), np.asarray(w_silu),
                                          np.asarray(w_sp))

    WP1bf = WP1.astype(ml_dtypes.bfloat16)
    bf = ml_dtypes.bfloat16
    BROW2 = np.tile((BROW.astype(np.float32) / 128.0).astype(bf), (128, 1))
    c2 = 6.0 ** -0.5
    sA = 2.0 ** (A_SH / 2)
    bias_vals = [2.0 * (7.0 - k) * c2 for _, k, _d in CUBES_R] + \
                [2.0 * (7.0 - k) * c2 * sA for _, k in CUBES_8]
    BIAS = np.tile(np.array(bias_vals, dtype=np.float32)[None, :], (128, 1))
    in_maps = []
    for c in range(NCORES):
        xT = np.ascontiguousarray(x[c * ROWS:(c + 1) * ROWS].T).astype(ml_dtypes.bfloat16)
        in_maps.append({"xT": xT, "Wp1": WP1bf, "Brow": BROW2,
                        "Bias": BIAS, "Wf0": WF[0], "Wf1": WF[1], "Wf2": WF[2],
                        "Wb01": WB[0:2], "Wb23": WB[2:4], "W8": W8})

    res = bass_utils.run_bass_kernel_spmd(
        nc, in_maps, core_ids=list(range(NCORES)), trace=TRACE
    )
    global LAST_RESULTS
    LAST_RESULTS = res
    return np.concatenate([res.results[c]["out"] for c in range(NCORES)], axis=0)


# revision 23
# speedup vs baseline: 1.0268x; 1.0254x over previous
"""KAN layer (uniform cubic B-spline, grid=8, k=3) Trainium2 kernel, v4.

Math (unchanged from v3)
------------------------
Per batch row n and output o:
    out[n,o] = sum_i w_silu[i,o]*silu(x[n,i]) + sum_i w_sp[i,o] * sum_b B_b(x[n,i]) * C[b,i,o]

With the uniform knot grid t_j = -1.75 + 0.25*j, put s = 4x+7 in [3,11). The
spline space restricted to [-1,1] is spanned by 11 truncated-power functions
{1, x, x^2, x^3} u {(s-k)_+^3/6 : k=7..10} u {(k-s)_+^3/6 : k=4..6} (the
two-sided split keeps every tile O(1)-bounded so reduced-precision matmul
stays well-conditioned). silu(x) is least-squares fitted in the SAME basis
(max abs fit err 2.9e-6) so w_silu folds into the 11 weight groups. The
device computes 11 activation tiles and contracts them against (n_in, n_out)
weight groups on the PE, fp32 PSUM. Precision mix (host-sim rel err 1.31e-2
vs 2e-2 budget): P0/P1 bf16, P2/P3/R7 f32r, L6/R8/L5/R9 bf16, L4+R10 packed
in ONE fp8e4m3 DoubleRow group (tiles 2^10, weights 2^12, second PSUM bank
set, merged as out = psum_main + 2^-22 * psum_fp8).

v4 schedule changes (v3 measured: ACT 33.4us busy vs PE-stream 33us; PE idle
0..7.9us; 1KB-line output DMAs issued late through a backed-up Sync queue)
---------------------------------------------------------------------------
- Each cube tile is now Square(ACT, scale/bias does the knot shift; the *4
  of u=s-k folded into the scale) -> stt(DVE, (x+(7-k)/4)*t2) -> relu as a
  DVE tensor_scalar max(q,0) [R side] / min(q,0) [L side, weights negated on
  host] - the 7 ACT Relu passes (14us) move to DVE at 0.6us each, and the
  f32 U=4x+7 helper tile disappears (stt reads xt directly).
- p2 / p3 are single 2048-wide passes instead of 4x512.
- PE warm-up: the HAM clock gate needs ~3.4us of continuous PE work before
  it unthrottles 1.2->2.4 GHz; scratch matmuls + the 4 rank-1 psum seeds
  fill the DMA lead-in exactly, so real groups run at full clock.
- Group order P1,P2,P3,R7,L6,R8,L5,DR,R9: the fp8 group closes early so its
  ACT scale-copies overlap R9; R9 closes each psum bank last and its per-m
  completion chases a full-width DVE merge + one 2KB-line output DMA.
- One dma_start per weight plane, separate SBUF tiles per plane (exact
  dependency granularity), issued in first-use order.

Sharding: data-parallel over batch N across 8 cores (512 rows each);
weights replicated. No collectives.
"""

import numpy as np
import ml_dtypes

N, N_IN, N_OUT = 4096, 512, 512
NB = 11
NCORES = 8
ROWS = N // NCORES          # batch rows per core
G = N_IN // 128             # 4 partition groups over n_in
M = ROWS // 128             # 4 PSUM row-chunks
W5 = (1.0, -4.0, 6.0, -4.0, 1.0)

# silu(x) ~ lstsq fit in the truncated-power basis
# [1, x, x^2, x^3, L4, L5, L6, R7, R8, R9, R10]  (max abs err 2.9e-6)
SILU_COEF = np.array([
    -2.85017504e-06, 5.00000000e-01, 2.51316134e-01, 1.04215478e-02,
    -6.77741053e-04, -1.30248882e-03, -1.77424080e-03, -1.95404022e-03,
    -1.77424080e-03, -1.30248882e-03, -6.77741053e-04,
])

# full-group cubes in matmul-group order: (kind, knot, chain dtype)
CUBES_R = [("R", 7, "f32"), ("L", 6, "bf"), ("R", 8, "bf"), ("L", 5, "bf"), ("R", 9, "bf")]
CUBES_8 = [("L", 4), ("R", 10)]   # fp8 DoubleRow pair
A_SH = 10   # fp8 tile scale 2^A_SH
B_SH = 12   # fp8 weight scale 2^B_SH
N_SCRATCH = 52   # scratch warm-up matmuls before the 4 psum seeds

_CACHE = {}


def _fp32r(a):
    """Round float32 array to fp32r (11-bit mantissa, RNE)."""
    a = np.ascontiguousarray(a, dtype=np.float32)
    bits = a.view(np.uint32)
    rnd = ((bits >> np.uint32(12)) & np.uint32(1)) + np.uint32(0x7FF)
    return ((bits + rnd) & np.uint32(0xFFFFF000)).view(np.float32)


def _poly_alpha():
    """alpha[j, t]: coefficient of x^t in the polynomial part of B_j."""
    alpha = np.zeros((NB, 4), dtype=np.float64)
    for j in range(NB):
        for p in range(5):
            k = j + p
            if k <= 6:  # (s-k)^3/6 with s-k = 4x + (7-k)
                a = 7.0 - k
                alpha[j, 3] += W5[p] * 64.0 / 6.0
                alpha[j, 2] += W5[p] * 48.0 * a / 6.0
                alpha[j, 1] += W5[p] * 12.0 * a * a / 6.0
                alpha[j, 0] += W5[p] * a * a * a / 6.0
    return alpha


def _prep_weights(C, w_silu, w_sp):
    """Fold C*w_sp and the silu fit into the 11 weight groups.
    L-side cube tiles are produced as min(q,0) = -(k-s)_+^3/6 on device,
    so L weights are negated here."""
    Ceff = C.astype(np.float64) * w_sp.astype(np.float64)[None]
    ws = w_silu.astype(np.float64)
    alpha = _poly_alpha()
    beta = np.einsum("jt,jio->tio", alpha, Ceff)  # (4, n_in, n_out)
    Wp = [beta[t] + SILU_COEF[t] * ws for t in range(4)]

    cube_order = [("L", 4), ("L", 5), ("L", 6), ("R", 7), ("R", 8), ("R", 9), ("R", 10)]
    Wc = {}
    for gi, (kind, k) in enumerate(cube_order):
        wk = np.zeros((N_IN, N_OUT), dtype=np.float64)
        for p in range(5):
            j = k - p
            if 0 <= j < NB:
                wk += W5[p] * Ceff[j]
        wk = wk + SILU_COEF[4 + gi] * ws
        if kind == "L":
            wk = -wk          # device L tiles are min(q,0) <= 0
        Wc[(kind, k)] = wk

    bf = ml_dtypes.bfloat16
    BROW = Wp[0].sum(axis=0, keepdims=True).astype(np.float32).astype(bf)
    WP1 = Wp[1].astype(np.float32).astype(bf)
    WF = _fp32r(np.stack([Wp[2], Wp[3], Wc[("R", 7)]]).astype(np.float32))
    WB = np.stack([Wc[("L", 6)], Wc[("R", 8)], Wc[("L", 5)], Wc[("R", 9)]])
    WB = WB.astype(np.float32).astype(bf)
    W8 = np.empty((N_IN, 2, N_OUT), dtype=np.float32)
    W8[:, 0] = Wc[CUBES_8[0]] * 2.0 ** B_SH
    W8[:, 1] = Wc[CUBES_8[1]] * 2.0 ** B_SH
    W8 = W8.astype(ml_dtypes.float8_e4m3)
    return BROW, WP1, WF, WB, W8


def _build():
    import concourse.bacc as bacc
    import concourse.mybir as mybir
    from concourse import tile

    f32 = mybir.dt.float32
    f32r = mybir.dt.float32r
    bf16 = mybir.dt.bfloat16
    fp8 = mybir.dt.float8e4
    AF = mybir.ActivationFunctionType
    ALU = mybir.AluOpType
    DR = mybir.MatmulPerfMode.DoubleRow

    c2 = 6.0 ** (-0.5)          # sqrt scaling: t2 = (2*c2*(s-k))^2 = (2/3)u^2
    sA = 2.0 ** (A_SH / 2)      # extra sqrt scale for the fp8 pair

    nc = bacc.Bacc("TRN2", target_bir_lowering=False, debug=False)
    TXT = nc.dram_tensor("xT", [N_IN, ROWS], bf16, kind="ExternalInput").ap()
    TP1 = nc.dram_tensor("Wp1", [N_IN, N_OUT], bf16, kind="ExternalInput").ap()
    TB = nc.dram_tensor("Brow", [32, N_OUT], bf16, kind="ExternalInput").ap()
    NBIAS = len(CUBES_R) + len(CUBES_8)
    TBIAS = nc.dram_tensor("Bias", [128, NBIAS], f32, kind="ExternalInput").ap()
    TWF = [nc.dram_tensor(f"Wf{i}", [N_IN, N_OUT], f32r,
                          kind="ExternalInput").ap() for i in range(3)]
    TWB01 = nc.dram_tensor("Wb01", [2, N_IN, N_OUT], bf16, kind="ExternalInput").ap()
    TWB23 = nc.dram_tensor("Wb23", [2, N_IN, N_OUT], bf16, kind="ExternalInput").ap()
    TW8 = nc.dram_tensor("W8", [N_IN, 2, N_OUT], fp8, kind="ExternalInput").ap()
    OUT = nc.dram_tensor("out", [ROWS, N_OUT], f32, kind="ExternalOutput").ap()

    with tile.TileContext(nc) as tc:
        with (
            tc.tile_pool(name="const", bufs=1) as constp,
            tc.tile_pool(name="t2f", bufs=1) as t2fp,
            tc.tile_pool(name="t2b", bufs=3) as t2bp,
            tc.tile_pool(name="qf", bufs=1) as qfp,
            tc.tile_pool(name="qb", bufs=3) as qbp,
            tc.tile_pool(name="outp", bufs=6) as outp,
            tc.tile_pool(name="psp", bufs=1, space="PSUM") as psp,
        ):
            # ---- persistent tiles ----
            xt = constp.tile([128, G, ROWS], bf16)
            dqP1 = constp.tile([128, G, N_OUT], bf16)
            wrm = constp.tile([128, 128], mybir.dt.uint16)
            brow2 = constp.tile([32, N_OUT], bf16)    # brow/32 replicated
            bias_t2 = constp.tile([128, NBIAS], f32)
            dqF = [constp.tile([128, G, N_OUT], f32r, name=f"dqF{i}",
                                tag=f"dqF{i}") for i in range(3)]
            dqB01 = constp.tile([128, 2, G, N_OUT], bf16)
            dqB23 = constp.tile([128, 2, G, N_OUT], bf16)
            dq8 = constp.tile([128, G, 2, N_OUT], fp8)
            p2 = constp.tile([128, G, ROWS], f32r)
            p3 = constp.tile([128, G, ROWS], f32r)
            cube_tiles = []
            for ci, (kind, k, dt) in enumerate(CUBES_R):
                cube_tiles.append(constp.tile(
                    [128, G, ROWS], f32r if dt == "f32" else bf16,
                    name=f"cube{k}", tag=f"cube{k}"))
            cube8 = constp.tile([128, 2, G, ROWS], fp8)

            # ---- all DMAs up-front, bundled, first-use order.  brow is
            # FIRST: transfers drain FIFO, and the psum seed matmuls (which
            # precede P1 on the PE queue) block on it ----
            # bf16 memset fails the walrus ISA check; memset the bf16 1.0
            # bit-pattern into a uint16 tile and bitcast for the warm-ups
            nc.gpsimd.memset(wrm[:], 0x3F80)
            nc.sync.dma_start(brow2[:], TB[:])
            nc.sync.dma_start(xt[:], TXT.rearrange("(g p) n -> p g n", p=128))
            nc.sync.dma_start(dqP1[:], TP1.rearrange("(g p) o -> p g o", p=128))
            nc.sync.dma_start(bias_t2[:], TBIAS[:])
            for i in range(3):
                nc.sync.dma_start(dqF[i][:], TWF[i].rearrange("(g p) o -> p g o", p=128))
            nc.sync.dma_start(dq8[:], TW8.rearrange("(g p) two o -> p g two o", p=128))
            nc.sync.dma_start(dqB01[:], TWB01.rearrange("t (g p) o -> p t g o", p=128))
            nc.sync.dma_start(dqB23[:], TWB23.rearrange("t (g p) o -> p t g o", p=128))

            ot_all = constp.tile([128, M, N_OUT], f32)
            psm = [psp.tile([128, N_OUT], f32, name=f"ps{m}", tag=f"ps{m}") for m in range(M)]
            ps8 = [psp.tile([128, N_OUT], f32, name=f"q{m}", tag=f"q{m}") for m in range(M)]

            # ---- PE warm-up: fill the HAM window until Wp1's DMA lands.
            # Full 128-K matmuls: HAM's activity monitor ignores rank-1 work.
            # N=128 -> ~107ns each cold; the tiny wrm block lands first ----
            for _ in range(N_SCRATCH):
                nc.tensor.matmul(ps8[M - 1][:, 0:128], wrm[:].bitcast(bf16),
                                 wrm[:].bitcast(bf16), start=True, stop=True)
            # constant group seeds each psum bank: full-K ones x brow/128
            # (K=128 so the HAM activity monitor credits these too)
            for m in range(M):
                nc.tensor.matmul(psm[m][:], wrm[0:32, :].bitcast(bf16), brow2[:],
                                 start=True, stop=False)

            # ---- ACT queue: p2 then one Square per cube, in group order ----
            nc.scalar.activation(p2[:], xt[:], AF.Square)
            sq = {}

            def emit_sq(ci):
                kind, k, dt = CUBES_R[ci]
                pool = t2fp if dt == "f32" else t2bp
                t2 = pool.tile([128, G, ROWS], f32 if dt == "f32" else bf16,
                               name="t2", tag=f"t2{dt}")
                nc.scalar.activation(t2[:], xt[:], AF.Square,
                                     bias=bias_t2[:, ci:ci + 1], scale=8.0 * c2)
                sq[ci] = t2

            def emit_sq8(idx):
                t2 = t2bp.tile([128, G, ROWS], bf16, name="t2", tag="t2bf")
                nc.scalar.activation(t2[:], xt[:], AF.Square,
                                     bias=bias_t2[:, 5 + idx:5 + idx + 1],
                                     scale=8.0 * c2 * sA)
                sq[5 + idx] = t2

            emit_sq(0)            # R7 chain leads: its group follows P3
            emit_sq8(0)           # then the fp8 pair (DR group runs 5th)
            emit_sq8(1)
            for ci in (1, 2, 3, 4):
                emit_sq(ci)

            # ---- DVE queue: p3, then stt+relu per cube, in group order ----
            nc.vector.tensor_tensor(p3[:], p2[:], xt[:], op=ALU.mult)

            def emit_cube(kind, k, t2, dst, dt):
                pool = qfp if dt == "f32" else qbp
                q = pool.tile([128, G, ROWS], f32 if dt == "f32" else bf16,
                              name="q", tag=f"q{dt}")
                # q = (x + (7-k)/4) * t2 = u^3/6 (t2 carries the 4x scale)
                nc.vector.scalar_tensor_tensor(q[:], xt[:], (7.0 - k) / 4.0, t2[:],
                                               op0=ALU.add, op1=ALU.mult)
                # R: relu(q) = max(q,0); L: min(q,0) = -(k-s)_+^3/6 (W negated)
                nc.vector.tensor_scalar(dst, q[:], 0.0, None,
                                        op0=ALU.max if kind == "R" else ALU.min)

            emit_cube(*CUBES_R[0][:2], sq[0], cube_tiles[0][:], CUBES_R[0][2])
            for idx, (kind, k) in enumerate(CUBES_8):
                emit_cube(kind, k, sq[5 + idx], cube8[:, idx], "bf")
            for ci in (1, 2, 3, 4):
                kind, k, dt = CUBES_R[ci]
                emit_cube(kind, k, sq[ci], cube_tiles[ci][:], dt)

            # ---- matmul groups ----
            def emit_mm(lhs_of, dq_of, last=False):
                for m in range(M):
                    for g in range(G):
                        nc.tensor.matmul(
                            psm[m][:], lhs_of(m, g), dq_of(g),
                            start=False, stop=(last and g == G - 1),
                        )

            emit_mm(lambda m, g: xt[:, g, m * 128:(m + 1) * 128],
                    lambda g: dqP1[:, g, :])
            emit_mm(lambda m, g: p2[:, g, m * 128:(m + 1) * 128],
                    lambda g: dqF[0][:, g, :])
            emit_mm(lambda m, g: p3[:, g, m * 128:(m + 1) * 128],
                    lambda g: dqF[1][:, g, :])
            dq_of = [lambda g: dqF[2][:, g, :], lambda g: dqB01[:, 0, g, :],
                     lambda g: dqB01[:, 1, g, :], lambda g: dqB23[:, 0, g, :],
                     lambda g: dqB23[:, 1, g, :]]
            emit_mm(lambda m, g: cube_tiles[0][:, g, m * 128:(m + 1) * 128],
                    dq_of[0])                        # R7

            # fp8 DoubleRow pair into the second PSUM bank set; scale-copies
            # to SBUF chase each m so the R9 merges only touch one PSUM tile
            tmp8s = []
            for m in range(M):
                for g in range(G):
                    nc.tensor.matmul(
                        ps8[m][:],
                        cube8[:, :, g, m * 128:(m + 1) * 128],
                        dq8[:, g, :, :],
                        start=(g == 0), stop=(g == G - 1),
                        perf_mode=DR,
                    )
                tmp8 = outp.tile([128, N_OUT], f32, name="tmp8", tag="tmp8")
                nc.scalar.activation(tmp8[:], ps8[m][:], AF.Copy,
                                     scale=2.0 ** -(A_SH + B_SH))
                tmp8s.append(tmp8)

            for ci in (1, 2, 3):                     # L6, R8, L5
                cube = cube_tiles[ci]
                emit_mm(lambda m, g, cube=cube: cube[:, g, m * 128:(m + 1) * 128],
                        dq_of[ci])

            # last bf16 group (R9) closes the main accumulation; each m's
            # merge + full-width store chases its final matmul
            cube = cube_tiles[len(CUBES_R) - 1]
            for m in range(M):
                for g in range(G):
                    nc.tensor.matmul(
                        psm[m][:], cube[:, g, m * 128:(m + 1) * 128],
                        dq_of[len(CUBES_R) - 1](g),
                        start=False, stop=(g == G - 1),
                    )
                nc.vector.scalar_tensor_tensor(
                    ot_all[:, m, :], psm[m][:], 1.0, tmp8s[m][:],
                    op0=ALU.mult, op1=ALU.add,
                )
                nc.sync.dma_start(OUT[m * 128:(m + 1) * 128, :], ot_all[:, m, :])

    nc.compile()
    return nc


# test-harness knobs (the grader just calls kernel())
TRACE = False
LAST_RESULTS = None


def kernel(x, grid, C, w_silu, w_sp):
    from concourse import bass_utils

    if "nc" not in _CACHE:
        _CACHE["nc"] = _build()
    nc = _CACHE["nc"]

    x = np.ascontiguousarray(np.asarray(x, dtype=np.float32))
    BROW, WP1, WF, WB, W8 = _prep_weights(np.asarray(C), np.asarray(w_silu),
                                          np.asarray(w_sp))

    WP1bf = WP1.astype(ml_dtypes.bfloat16)
    bf = ml_dtypes.bfloat16
    BROW2 = np.tile((BROW.astype(np.float32) / 32.0).astype(bf), (32, 1))
    c2 = 6.0 ** -0.5
    sA = 2.0 ** (A_SH / 2)
    bias_vals = [2.0 * (7.0 - k) * c2 for _, k, _d in CUBES_R] + \
                [2.0 * (7.0 - k) * c2 * sA for _, k in CUBES_8]
    BIAS = np.tile(np.array(bias_vals, dtype=np.float32)[None, :], (128, 1))
    in_maps = []
    for c in range(NCORES):
        xT = np.ascontiguousarray(x[c * ROWS:(c + 1) * ROWS].T).astype(ml_dtypes.bfloat16)
        in_maps.append({"xT": xT, "Wp1": WP1bf, "Brow": BROW2,
                        "Bias": BIAS, "Wf0": WF[0], "Wf1": WF[1], "Wf2": WF[2],
                        "Wb01": WB[0:2], "Wb23": WB[2:4], "W8": W8})

    res = bass_utils.run_bass_kernel_spmd(
        nc, in_maps, core_ids=list(range(NCORES)), trace=TRACE
    )
    global LAST_RESULTS
    LAST_RESULTS = res
    return np.concatenate([res.results[c]["out"] for c in range(NCORES)], axis=0)
